# revision 16
# baseline (speedup 1.0000x reference)
"""Trainium2 Bass kernel for nn_CausalSelfAttention_12283606468211 (v2).

Sliding-window causal GQA attention (B=4, T=2048, C=1024, 16 q-heads,
4 kv-heads, head_dim 64, window 1024) with value-embedding gating,
RoPE + QK-RMSNorm, and output projection.

Sharding: 8 cores = 4 batches x 2 head-halves (identical SPMD program,
per-core differences carried by host-side input slicing).

v2 design (cost-model-driven):
 - fp8e4m3 + DoubleRow matmuls for QKV projections, PV and the output
   projection.  Weights are scaled x16 on the host; the scale cancels
   through QK-RMSNorm, the softmax denominator (ones vector = 256) and
   the x16 output-projection weights.
 - scores bf16: out[keys, q] per 128-key chunk; causal/window triangles
   and the dead half-blocks of partial chunks are initialized by
   constant-matrix matmuls so Exp runs as one wide op per PSUM segment.
   The 1/sqrt(d) scale rides on Exp's scale operand.
 - PV flipped: out[q, vdim] = probs_chunk.T @ v_chunk; softmax
   denominators land per-partition (1-column matmuls against a constant
   vector), normalization is a per-partition broadcast multiply.
 - RMS-norm sums of squares on DVE (mult + grouped reduce), not ScalarE.
 - phase 1 (qkv/rope/norm, DVE-heavy) is interleaved with phase 2
   (attention, ScalarE-heavy) so no engine sits idle for a whole phase:
   token tiles 0-5 are emitted up front, then two more per group.
 - exchange payload is the fp8 normalized attention output in
   [ydim, token] layout, one AllGather per 256-token group.
"""

import sys

sys.path.insert(0, "/opt/trn_rl_repo")

import numpy as np
import ml_dtypes

import concourse.bass as bass
import concourse.mybir as mybir
from concourse import bacc
from concourse.tile import TileContext
from concourse.bass_utils import run_bass_kernel_spmd

F32 = mybir.dt.float32
BF16 = mybir.dt.bfloat16
F8 = mybir.dt.float8e4
U32 = mybir.dt.uint32
AF = mybir.ActivationFunctionType
ALU = mybir.AluOpType
DR = mybir.MatmulPerfMode.DoubleRow
AX = mybir.AxisListType

B, T, C = 4, 2048, 1024
WINDOW = 1024
EPS = 1.1920928955078125e-07
MAGIC = 0x5F3759DF

USE_FP8 = False
PROJ_F8 = False
DT8 = F8 if USE_FP8 else BF16
WSCALE = 16.0 if USE_FP8 else 1.0
ONESV = 1.0
WPSCALE = 1.0
EXP_BIAS = -4.0 if USE_FP8 else 0.0
PT_F8 = False

NH = 8
NKV = 2
D = 64
QDIM = NH * D    # 512
KVDIM = NKV * D  # 128
NT = T // 128    # 16
NG = T // 256    # 8

_CACHE = {}


def _qsel(c, g):
    lo_t = max(2 * g, c)
    hi_t = min(2 * g + 1, c + 8)
    if lo_t > hi_t:
        return None
    return ((lo_t - 2 * g) * 128, (hi_t + 1 - 2 * g) * 128)


def _group_layout(g):
    """Union chunks of group g placed in 512-wide (2-slot) PSUM segment
    tiles.  placement[c] = (tile_idx, slot); chunk c's qtile-K columns
    live at slot*256 + K*128 of its tile.  Softmax denominators ride in
    the PV PSUM via a ones-column appended to v (no dn columns here)."""
    cs = list(range(max(0, 2 * g - 8), 2 * g + 2))
    n = len(cs)
    assert n % 2 == 0
    placement = {c: (i // 2, i % 2) for i, c in enumerate(cs)}
    ntiles = n // 2
    widths = [512] * ntiles
    # cl (and, for full windows, cf) contribute only one live 128-col
    # q-tile; pack them right after their partner chunk so the exp can
    # stop at 384 and the dead quarter is never touched.
    if n == 10:
        placement[cs[0]] = (0, 1)
        placement[cs[1]] = (0, 0)
        widths[0] = 384
    widths[-1] = 384
    return cs, placement, ntiles, widths


def build_program(fake_collective=False):
    nc = bacc.Bacc("TRN2", target_bir_lowering=False, debug=False,
                   enable_asserts=True, num_devices=8)

    xt_d = nc.dram_tensor("xt8", [128, 8, T], DT8, kind="ExternalInput")
    ve_d = nc.dram_tensor("ve_bf", [T, KVDIM], BF16, kind="ExternalInput")
    wq_d = nc.dram_tensor("wq8", [128, 8, QDIM], DT8, kind="ExternalInput")
    wkvz_d = nc.dram_tensor("wkvz8", [128, 8, 2 * KVDIM + NKV], DT8,
                            kind="ExternalInput")
    wp_d = nc.dram_tensor("wp8", [128, 8, 512], BF16, kind="ExternalInput")
    c2_d = nc.dram_tensor("c2_bf", [T, D], BF16, kind="ExternalInput")
    s2_d = nc.dram_tensor("s2_bf", [T, D], BF16, kind="ExternalInput")
    y_d = nc.dram_tensor("y_out", [T, 512], F32, kind="ExternalOutput")

    with TileContext(nc) as tc, nc.allow_low_precision(reason="fp8 pipeline"):
        with (
            tc.tile_pool(name="const", bufs=1) as constp,
            tc.tile_pool(name="persist", bufs=1) as pers,
            tc.tile_pool(name="work", bufs=3) as work,
            tc.tile_pool(name="probs", bufs=10) as probp,
            tc.tile_pool(name="att2", bufs=3) as att2,
            tc.tile_pool(name="ygsp", bufs=3) as ygsp,
            tc.tile_pool(name="p1ps", bufs=2, space="PSUM") as p1psp,
            tc.tile_pool(name="scps", bufs=2, space="PSUM") as scpsp,
            tc.tile_pool(name="b1ps", bufs=2, space="PSUM") as b1psp,
            tc.tile_pool(name="dram2", bufs=3, space="DRAM") as dram2p,
        ):
            # ---- constants ----
            ident = constp.tile([128, 128], BF16, name="ident")
            nc.gpsimd.memset(ident[:, :], 0.0)
            nc.gpsimd.affine_select(
                out=ident[:, :], in_=ident[:, :],
                compare_op=ALU.not_equal, fill=1.0, base=0,
                pattern=[[-1, 128]], channel_multiplier=1)
            # mask constants (rhs of identity-lhsT matmuls, direct
            # orientation psum[tk, tq] += M[tk, tq]):
            #   mm[:,   0:128] far  : -1e9 where tk <= tq
            #   mm[:, 128:384] all -1e9 (dead half-blocks)
            #   mm[:, 384:512] diag : -1e9 where tk > tq
            mm = constp.tile([128, 512], BF16, name="mm")
            nc.gpsimd.memset(mm[:, :], -1.0e9)
            nc.gpsimd.affine_select(
                out=mm[:, 0:128], in_=mm[:, 0:128],
                compare_op=ALU.is_ge, fill=0.0, base=0,
                pattern=[[1, 128]], channel_multiplier=-1)
            nc.gpsimd.affine_select(
                out=mm[:, 384:512], in_=mm[:, 384:512],
                compare_op=ALU.is_ge, fill=0.0, base=-1,
                pattern=[[-1, 128]], channel_multiplier=1)
            MFARF = mm[:, 0:256]
            MDIAGF = mm[:, 256:512]
            MDIAG = mm[:, 384:512]
            MFAR = mm[:, 0:128]
            ident8 = constp.tile([128, 128], DT8, name="ident8")
            nc.gpsimd.memset(ident8[:, :], 0.0)
            nc.gpsimd.affine_select(
                out=ident8[:, :], in_=ident8[:, :],
                compare_op=ALU.not_equal, fill=1.0, base=0,
                pattern=[[-1, 128]], channel_multiplier=1)
            ebias = constp.tile([128, 1], F32, name="ebias")
            nc.gpsimd.memset(ebias[:, :], EXP_BIAS)

            # ---- weights / tables ----
            xt_sb = pers.tile([128, 8, T], DT8, name="xt_sb")
            wq_sb = pers.tile([128, 8, QDIM], DT8, name="wq_sb")
            wkvz_sb = pers.tile([128, 8, 2 * KVDIM + NKV], DT8,
                                name="wkvz_sb")
            wp_sb = pers.tile([128, 8, 512], BF16, name="wp_sb")
            c2_sb = pers.tile([128, NT, D], BF16, name="c2_sb")
            s2_sb = pers.tile([128, NT, D], BF16, name="s2_sb")
            ve_sb = pers.tile([128, NT, KVDIM], BF16, name="ve_sb")
            nc.sync.dma_start(xt_sb[:, :, 0:512], xt_d.ap()[:, :, 0:512])
            nc.sync.dma_start(wq_sb[:, :, :], wq_d.ap())
            nc.sync.dma_start(wkvz_sb[:, :, :], wkvz_d.ap())
            nc.sync.dma_start(c2_sb[:, :, :],
                              c2_d.ap().rearrange("(t p) d -> p t d", p=128))
            nc.sync.dma_start(s2_sb[:, :, :],
                              s2_d.ap().rearrange("(t p) d -> p t d", p=128))
            nc.sync.dma_start(ve_sb[:, :, :],
                              ve_d.ap().rearrange("(t p) d -> p t d", p=128))
            for tq in range(1, 4):
                nc.sync.dma_start(xt_sb[:, :, tq * 512:(tq + 1) * 512],
                                  xt_d.ap()[:, :, tq * 512:(tq + 1) * 512])
            nc.sync.dma_start(wp_sb[:, :, :], wp_d.ap())

            qT_all = pers.tile([128, 4, NT, 128], BF16, name="qT_all")
            kT_all = pers.tile([128, NT, 128], BF16, name="kT_all")
            # v tiles carry a trailing ones-column (col D) so the PV
            # matmul also accumulates the softmax denominator.
            v_sb2 = [pers.tile([128, 2, NKV, D + 1], BF16, name=f"v2_{cp}")
                     for cp in range(NT // 2)]
            for cp in range(NT // 2):
                nc.gpsimd.memset(v_sb2[cp][:, :, :, D:D + 1], ONESV)

            # ================= phase-1 tile emitter =================
            def emit_tile(t):
                part_b = emit_tile_a(t)
                part_b()

            def emit_tile_a(t):
                q_ps = p1psp.tile([128, QDIM], F32, tag="p1",
                                  name=f"q_ps{t}")
                kvz = p1psp.tile([128, KVDIM * 2 + NKV], F32, tag="p1",
                                 name=f"kvz{t}")
                for cc in range(0, 8, 2):
                    lhs = xt_sb[:, cc:cc + 2, t * 128:(t + 1) * 128]
                    st, sp_ = (cc == 0), (cc == 6)
                    if USE_FP8:
                        nc.tensor.matmul(q_ps[:, :], lhs,
                                         wq_sb[:, cc:cc + 2, :],
                                         start=st, stop=sp_, perf_mode=DR)
                        nc.tensor.matmul(kvz[:, :], lhs,
                                         wkvz_sb[:, cc:cc + 2, :],
                                         start=st, stop=sp_, perf_mode=DR)
                    else:
                        for o in range(2):
                            l1 = xt_sb[:, cc + o, t * 128:(t + 1) * 128]
                            s1_, p1_ = (cc + o == 0), (cc + o == 7)
                            nc.tensor.matmul(q_ps[:, :], l1,
                                             wq_sb[:, cc + o, :],
                                             start=s1_, stop=p1_)
                            nc.tensor.matmul(kvz[:, :], l1,
                                             wkvz_sb[:, cc + o, :],
                                             start=s1_, stop=p1_)

                # -- early kvz consumers so the next tile's k/v matmuls
                # aren't held back long --
                e_sb = work.tile([128, NKV], F32, tag="e_sb")
                nc.scalar.activation(e_sb[:, :],
                                     kvz[:, 2 * KVDIM:2 * KVDIM + NKV],
                                     AF.Exp, bias=0.0, scale=-1.0 / WSCALE)
                gp1 = work.tile([128, NKV], F32, tag="gp1")
                nc.vector.tensor_scalar_add(gp1[:, :], e_sb[:, :], 1.0)
                gr = work.tile([128, NKV], F32, tag="gr")
                nc.vector.reciprocal(gr[:, :], gp1[:, :])
                # v' = (ve2 * gate) + v   (ve2 = 2*WSCALE*ve host-side)
                vdst = v_sb2[t // 2]
                for u in range(NKV):
                    nc.vector.scalar_tensor_tensor(
                        vdst[:, t % 2, u, 0:D],
                        ve_sb[:, t, u * D:(u + 1) * D],
                        gr[:, u:u + 1],
                        kvz[:, KVDIM + u * D:KVDIM + (u + 1) * D],
                        ALU.mult, ALU.add)
                k_sb = work.tile([128, KVDIM], BF16, tag="k_sb")
                nc.vector.tensor_copy(k_sb[:, :], kvz[:, 0:KVDIM])
                q_sb = work.tile([128, QDIM], BF16, tag="q_sb")
                nc.scalar.copy(q_sb[:, :], q_ps[:, :])

                # -- mean-square sums (RoPE is orthogonal: pre-rotation).
                # Squares on ScalarE straight from PSUM; reduce on DVE. --
                ms = work.tile([128, NH + NKV], F32, tag="ms")
                sqq = work.tile([128, QDIM], BF16, tag="sqq")
                nc.scalar.square(sqq[:, :], q_ps[:, :])
                nc.vector.tensor_reduce(
                    ms[:, 0:NH],
                    sqq[:, :].rearrange("p (h d) -> p h d", h=NH),
                    AX.X, ALU.add)
                sqk = work.tile([128, KVDIM], BF16, tag="sqk")
                nc.scalar.square(sqk[:, :], kvz[:, 0:KVDIM])
                nc.vector.tensor_reduce(
                    ms[:, NH:NH + NKV],
                    sqk[:, :].rearrange("p (u d) -> p u d", u=NKV),
                    AX.X, ALU.add)

                # rs = 1/sqrt(ms + eps'): Sqrt on ScalarE, reciprocal on DVE
                NR = NH + NKV
                a = work.tile([128, NR], F32, tag="a")
                nc.vector.tensor_scalar_add(a[:, :], ms[:, :],
                                            64.0 * EPS * WSCALE * WSCALE)
                sq = work.tile([128, NR], F32, tag="sq")
                nc.scalar.sqrt(sq[:, :], a[:, :])
                rs = work.tile([128, NR], F32, tag="rs")
                nc.vector.reciprocal(rs[:, :], sq[:, :])

                def part_b(t=t, q_sb=q_sb, k_sb=k_sb, rs=rs):
                    emit_tile_b(t, q_sb, k_sb, rs)
                return part_b

            def emit_tile_b(t, q_sb, k_sb, rs):
                # -- q rope: half-products on Pool, combine on DVE --
                qg = q_sb[:, :].rearrange("p (h s f) -> p h s f", h=NH, s=2)
                c2b = c2_sb[:, t].unsqueeze(1).broadcast_to([128, NH, D])
                s2b = s2_sb[:, t].rearrange("p (s f) -> p s f", s=2)
                m1 = work.tile([128, QDIM], BF16, tag="m1")
                m1g = m1[:, :].rearrange("p (h s f) -> p h s f", h=NH, s=2)
                nc.gpsimd.tensor_mul(
                    m1g[:, :, 0], qg[:, :, 1],
                    s2b[:, 0].unsqueeze(1).broadcast_to([128, NH, 32]))
                nc.gpsimd.tensor_mul(
                    m1g[:, :, 1], qg[:, :, 0],
                    s2b[:, 1].unsqueeze(1).broadcast_to([128, NH, 32]))
                rq = work.tile([128, QDIM], BF16, tag="rq")
                nc.vector.tensor_mul(
                    rq[:, :].rearrange("p (h d) -> p h d", h=NH),
                    q_sb[:, :].rearrange("p (h d) -> p h d", h=NH), c2b)
                nc.vector.tensor_add(rq[:, :], rq[:, :], m1[:, :])

                # q~ = rq * rs, head j -> column (j%4)*128 + (j//4)*64
                qn = work.tile([128, QDIM], BF16, tag="qn")
                nc.vector.tensor_tensor(
                    qn[:, :].rearrange("p (m hh d) -> p hh m d", m=4, hh=2),
                    rq[:, :].rearrange("p (hh m d) -> p hh m d", hh=2, m=4),
                    rs[:, 0:NH].rearrange("p (hh m) -> p hh m", hh=2)
                    .unsqueeze(3).broadcast_to([128, 2, 4, D]),
                    ALU.mult)
                nc.sync.dma_start_transpose(
                    out=qT_all[:, :, t, :], in_=qn[:, :])

                # -- k rope (Pool) --
                kg = k_sb[:, :].rearrange("p (u s f) -> p u s f", u=NKV, s=2)
                km = work.tile([128, KVDIM], BF16, tag="km")
                kmg = km[:, :].rearrange("p (u s f) -> p u s f", u=NKV, s=2)
                nc.gpsimd.tensor_mul(
                    kmg[:, :, 0], kg[:, :, 1],
                    s2b[:, 0].unsqueeze(1).broadcast_to([128, NKV, 32]))
                nc.gpsimd.tensor_mul(
                    kmg[:, :, 1], kg[:, :, 0],
                    s2b[:, 1].unsqueeze(1).broadcast_to([128, NKV, 32]))
                rk = work.tile([128, KVDIM], BF16, tag="rk")
                nc.gpsimd.tensor_mul(
                    rk[:, :].rearrange("p (u d) -> p u d", u=NKV),
                    k_sb[:, :].rearrange("p (u d) -> p u d", u=NKV),
                    c2_sb[:, t].unsqueeze(1).broadcast_to([128, NKV, D]))
                nc.gpsimd.tensor_add(rk[:, :], rk[:, :], km[:, :])
                # k~ = rk * rs_k (the 1/sqrt(d)=x8 scale rides on Exp)
                krs = work.tile([128, KVDIM], BF16, tag="krs")
                nc.vector.tensor_tensor(
                    krs[:, :].rearrange("p (u d) -> p u d", u=NKV),
                    rk[:, :].rearrange("p (u d) -> p u d", u=NKV),
                    rs[:, NH:NH + NKV].unsqueeze(2)
                    .broadcast_to([128, NKV, D]),
                    ALU.mult)
                nc.sync.dma_start_transpose(out=kT_all[:, t, :],
                                            in_=krs[:, :])

            # ================= phase-2 group machinery =================
            def emit_scores_tile(g, j, ti, cs, placement, widths, cf,
                                 cl, pts):
                base = (j // 4) * 64
                m = j % 4
                sc = scpsp.tile([128, 512], F32, tag="sc",
                                name=f"sc{g}_{j}_{ti}")
                for c in cs:
                    tic, s = placement[c]
                    if tic != ti:
                        continue
                    o = s * 256
                    if c == cf:
                        nc.tensor.matmul(sc[:, o:o + 128], ident[:, :],
                                         MFAR, start=True, stop=False,
                                         skip_group_check=True)
                        nc.tensor.matmul(
                            sc[:, o:o + 128],
                            kT_all[base:base + 64, c, :],
                            qT_all[base:base + 64, m, 2 * g, :],
                            start=False, stop=True, skip_group_check=True)
                    elif c == cl:
                        nc.tensor.matmul(sc[:, o:o + 128], ident[:, :],
                                         MDIAG, start=True, stop=False,
                                         skip_group_check=True)
                        nc.tensor.matmul(
                            sc[:, o:o + 128],
                            kT_all[base:base + 64, c, :],
                            qT_all[base:base + 64, m, 2 * g + 1, :],
                            start=False, stop=True, skip_group_check=True)
                    else:
                        diag = (c == 2 * g)
                        far1 = (c == 2 * g - 7)
                        nc.tensor.matmul(
                            sc[:, o:o + 256],
                            kT_all[base:base + 64, c, :],
                            qT_all[base:base + 64, m, 2 * g:2 * g + 2, :],
                            start=True, stop=not (diag or far1),
                            skip_group_check=True)
                        if diag:
                            nc.tensor.matmul(
                                sc[:, o:o + 128], ident[:, :], MDIAG,
                                start=False, stop=True,
                                skip_group_check=True)
                        if far1:
                            nc.tensor.matmul(
                                sc[:, o + 128:o + 256], ident[:, :], MFAR,
                                start=False, stop=True,
                                skip_group_check=True)
                pt = probp.tile([128, 512], BF16, tag="pt",
                                name=f"pt{g}_{j}_{ti}")
                nc.scalar.activation(pt[:, 0:widths[ti]],
                                     sc[:, 0:widths[ti]],
                                     AF.Exp, bias=ebias[:, :], scale=8.0)
                pts.setdefault(j, []).append(pt)

            def emit_pv(g, j, cs, placement, pts, yt8):
                u = j // 4
                jj = j % 4
                for K in range(2):
                    t = 2 * g + K
                    ck = [c for c in cs if max(0, t - 8) <= c <= t]
                    ops = []
                    i = 0
                    while i < len(ck):
                        c = ck[i]
                        if PT_F8 and c % 2 == 0 and i + 1 < len(ck):
                            ops.append((True, c))
                            i += 2
                        else:
                            ops.append((False, c))
                            i += 1
                    nops = len(ops)
                    dst = yt8[K][u][:, jj * (D + 1):(jj + 1) * (D + 1)]
                    for oi, (paired, c) in enumerate(ops):
                        st, sp_ = (oi == 0), (oi == nops - 1)
                        ti, s = placement[c]
                        pt = pts[j][ti]
                        if paired:
                            lhs = pt[:, :].rearrange(
                                "p (s q) -> p s q", s=2)[
                                :, :, K * 128:(K + 1) * 128]
                            nc.tensor.matmul(
                                dst, lhs,
                                v_sb2[c // 2][:, :, u, :],
                                start=st, stop=sp_, perf_mode=DR,
                                skip_group_check=True)
                        else:
                            # cl packs its single live q-tile at o+0
                            o = (s % 2) * 256 + (0 if c == 2 * g + 1
                                                 else K * 128)
                            lhs = pt[:, o:o + 128]
                            nc.tensor.matmul(
                                dst, lhs,
                                v_sb2[c // 2][:, c % 2, u, :],
                                start=st, stop=sp_, skip_group_check=True)

            def emit_finish(g, yt8):
                """normalize + transpose + exchange for group g; returns
                a closure emitting the projection (deferred so the
                exchange round-trip hides under the next group)."""
                rec = att2.tile([128, 2, 2, 4], F32, tag="rec",
                                name=f"rec{g}")
                for K in range(2):
                    for u in range(2):
                        nc.vector.reciprocal(
                            rec[:, K, u, :],
                            yt8[K][u][:, :].rearrange(
                                "p (h e) -> p h e", e=D + 1)[:, :, D])
                agin = dram2p.tile([8, 128, 128], BF16, tag="agin")
                agout = dram2p.tile([16, 128, 128], BF16, tag="agout")
                for K in range(2):
                    yn = att2.tile([128, QDIM], BF16, tag="yn",
                                   name=f"yn{g}_{K}")
                    for u in range(2):
                        nc.vector.tensor_tensor(
                            yn[:, u * 256:(u + 1) * 256].rearrange(
                                "p (h d) -> p h d", h=4),
                            yt8[K][u][:, :].rearrange(
                                "p (h e) -> p h e", e=D + 1)[:, :, 0:D],
                            rec[:, K, u, :].unsqueeze(2)
                            .broadcast_to([128, 4, D]),
                            ALU.mult)
                    ytp = b1psp.tile([128, QDIM], BF16, tag="b1",
                                     bufs=4, name=f"ytp{g}_{K}")
                    for bl in range(4):
                        nc.tensor.transpose(
                            ytp[:, bl * 128:(bl + 1) * 128],
                            yn[:, bl * 128:(bl + 1) * 128], ident[:, :])
                    yto = att2.tile([128, QDIM], BF16, tag="yto",
                                    name=f"yto{g}_{K}")
                    nc.vector.tensor_copy(yto[:, :], ytp[:, :])
                    nc.sync.dma_start(
                        agin[K * 4:(K + 1) * 4, :, :].rearrange(
                            "b p n -> p b n"),
                        yto[:, :].rearrange("p (b n) -> p b n", b=4))
                if fake_collective:
                    nc.sync.dma_start(agout[0:8], agin[:, :, :])
                    nc.sync.dma_start(agout[8:16], agin[:, :, :])
                else:
                    nc.gpsimd.collective_compute(
                        "AllGather", ALU.bypass,
                        replica_groups=[[0, 1], [2, 3], [4, 5], [6, 7]],
                        ins=[agin[:, :, :].opt()],
                        outs=[agout[:, :, :].opt()])
                ygs = ygsp.tile([128, 16, 128], BF16, tag="ygs",
                                name=f"ygs{g}")
                nc.sync.dma_start(
                    ygs[:, :, :],
                    agout[:, :, :].rearrange("c p n -> p c n"))

                def proj():
                    o2 = att2.tile([128, 2, 512], F32, tag="o2",
                                   name=f"o2_{g}")
                    for K in range(2):
                        # chunk order in ygs: [own t0 | own t1 | peer t0
                        # | peer t1], 4 blocks each
                        idx = [K * 4, K * 4 + 8]
                        pr = scpsp.tile([128, 512], F32, tag="sc",
                                        name=f"pr{g}_{K}")
                        if PROJ_F8:
                            for hi, i0 in enumerate(idx):
                                for c2i in range(0, 4, 2):
                                    nc.tensor.matmul(
                                        pr[:, :],
                                        ygs[:, i0 + c2i:i0 + c2i + 2, :],
                                        wp_sb[:, hi * 4 + c2i:
                                              hi * 4 + c2i + 2, :],
                                        start=(hi == 0 and c2i == 0),
                                        stop=(hi == 1 and c2i == 2),
                                        perf_mode=DR,
                                        skip_group_check=True)
                        else:
                            for hi, i0 in enumerate(idx):
                                for c1 in range(4):
                                    nc.tensor.matmul(
                                        pr[:, :], ygs[:, i0 + c1, :],
                                        wp_sb[:, hi * 4 + c1, :],
                                        start=(hi == 0 and c1 == 0),
                                        stop=(hi == 1 and c1 == 3),
                                        skip_group_check=True)
                        nc.vector.tensor_copy(o2[:, K, :], pr[:, :])
                    nc.sync.dma_start(
                        y_d.ap()[g * 256:(g + 1) * 256, :].rearrange(
                            "(i p) n -> p i n", p=128),
                        o2[:, :, :])
                return proj

            # ================= interleaved schedule =================
            for t in range(8):
                emit_tile(t)
            deferred_proj = None
            for g in range(NG):
                cs, placement, ntiles, widths = _group_layout(g)
                cf = cs[0] if _qsel(cs[0], g) == (0, 128) else None
                cl = cs[-1]
                yt8 = [[b1psp.tile([128, 4 * (D + 1)], F32, tag="b1",
                                   bufs=4, name=f"yt8_{g}_{K}_{u}")
                        for u in range(2)] for K in range(2)]
                pts = {}
                for j in range(NH):
                    emit_scores_tile(g, j, 0, cs, placement, widths,
                                     cf, cl, pts)
                    if j > 0:
                        emit_pv(g, j - 1, cs, placement, pts, yt8)
                    for ti in range(1, ntiles):
                        emit_scores_tile(g, j, ti, cs, placement, widths,
                                         cf, cl, pts)
                    if j == 5 and deferred_proj is not None:
                        deferred_proj()
                        deferred_proj = None
                    if j == 1 and 2 * g + 8 < NT:
                        emit_tile(2 * g + 8)
                    if j == 3 and 2 * g + 9 < NT:
                        emit_tile(2 * g + 9)
                emit_pv(g, NH - 1, cs, placement, pts, yt8)
                deferred_proj = emit_finish(g, yt8)
            deferred_proj()

    nc.compile()
    return nc


def _prep_inputs(x, ve, cos, sin, wq, wk, wv, wproj, wgate):
    bf = ml_dtypes.bfloat16
    f8 = ml_dtypes.float8_e4m3
    w8 = f8 if USE_FP8 else bf
    cosf = np.asarray(cos, np.float32).reshape(T, 32)
    sinf = np.asarray(sin, np.float32).reshape(T, 32)
    c2 = np.concatenate([cosf, cosf], axis=1).astype(bf)
    s2 = np.concatenate([sinf, -sinf], axis=1).astype(bf)
    x = np.asarray(x, np.float32)
    ve = np.asarray(ve, np.float32)
    wq = np.asarray(wq, np.float32) * WSCALE
    wk = np.asarray(wk, np.float32) * WSCALE
    wv = np.asarray(wv, np.float32) * WSCALE
    wproj = np.asarray(wproj, np.float32) * WPSCALE
    wgate = np.asarray(wgate, np.float32) * WSCALE

    def wfmt(w):  # [1024, N] -> [128, 8, N]
        return np.ascontiguousarray(
            w.reshape(8, 128, -1).transpose(1, 0, 2)).astype(w8)

    maps = []
    for core in range(8):
        b, hp = core // 2, core % 2
        xt8 = np.ascontiguousarray(
            x[b].T.reshape(8, 128, T).transpose(1, 0, 2)).astype(w8)
        wgp = np.zeros((1024, NKV), np.float32)
        wgp[0:8] = wgate[:, hp * 2:(hp + 1) * 2]
        wkvz = np.concatenate([wk[:, hp * 128:(hp + 1) * 128],
                               wv[:, hp * 128:(hp + 1) * 128], wgp], axis=1)
        maps.append({
            "xt8": xt8,
            "ve_bf": (2.0 * WSCALE
                      * ve[b][:, hp * 128:(hp + 1) * 128]).astype(bf),
            "wq8": wfmt(wq[:, hp * 512:(hp + 1) * 512]),
            "wkvz8": wfmt(wkvz),
            "wp8": np.ascontiguousarray(
                wproj[:, hp * 512:(hp + 1) * 512].reshape(8, 128, -1)
                .transpose(1, 0, 2)).astype(bf),
            "c2_bf": c2,
            "s2_bf": s2,
        })
    return maps


def kernel(x, ve, cos, sin, wq, wk, wv, wproj, wgate, window):
    assert int(window) == WINDOW
    if "nc" not in _CACHE:
        _CACHE["nc"] = build_program()
    nc = _CACHE["nc"]
    maps = _prep_inputs(x, ve, cos, sin, wq, wk, wv, wproj, wgate)
    res = run_bass_kernel_spmd(nc, maps, list(range(8))).results
    y = np.empty((B, T, C), np.float32)
    for core in range(8):
        b, hp = core // 2, core % 2
        y[b][:, hp * 512:(hp + 1) * 512] = res[core]["y_out"]
    return y



# revision 18
# speedup vs baseline: 1.0869x; 1.0869x over previous
"""Trainium2 Bass kernel for nn_CausalSelfAttention_12283606468211 (v2).

Sliding-window causal GQA attention (B=4, T=2048, C=1024, 16 q-heads,
4 kv-heads, head_dim 64, window 1024) with value-embedding gating,
RoPE + QK-RMSNorm, and output projection.

Sharding: 8 cores = 4 batches x 2 head-halves (identical SPMD program,
per-core differences carried by host-side input slicing).

v2 design (cost-model-driven):
 - fp8e4m3 + DoubleRow matmuls for QKV projections, PV and the output
   projection.  Weights are scaled x16 on the host; the scale cancels
   through QK-RMSNorm, the softmax denominator (ones vector = 256) and
   the x16 output-projection weights.
 - scores bf16: out[keys, q] per 128-key chunk; causal/window triangles
   and the dead half-blocks of partial chunks are initialized by
   constant-matrix matmuls so Exp runs as one wide op per PSUM segment.
   The 1/sqrt(d) scale rides on Exp's scale operand.
 - PV flipped: out[q, vdim] = probs_chunk.T @ v_chunk; softmax
   denominators land per-partition (1-column matmuls against a constant
   vector), normalization is a per-partition broadcast multiply.
 - RMS-norm sums of squares on DVE (mult + grouped reduce), not ScalarE.
 - phase 1 (qkv/rope/norm, DVE-heavy) is interleaved with phase 2
   (attention, ScalarE-heavy) so no engine sits idle for a whole phase:
   token tiles 0-5 are emitted up front, then two more per group.
 - exchange payload is the fp8 normalized attention output in
   [ydim, token] layout, one AllGather per 256-token group.
"""

import sys

sys.path.insert(0, "/opt/trn_rl_repo")

import numpy as np
import ml_dtypes

import concourse.bass as bass
import concourse.mybir as mybir
from concourse import bacc
from concourse.tile import TileContext
from concourse.bass_utils import run_bass_kernel_spmd

F32 = mybir.dt.float32
BF16 = mybir.dt.bfloat16
F8 = mybir.dt.float8e4
U32 = mybir.dt.uint32
AF = mybir.ActivationFunctionType
ALU = mybir.AluOpType
DR = mybir.MatmulPerfMode.DoubleRow
AX = mybir.AxisListType

B, T, C = 4, 2048, 1024
WINDOW = 1024
EPS = 1.1920928955078125e-07
MAGIC = 0x5F3759DF

USE_FP8 = False
PROJ_F8 = False
DT8 = F8 if USE_FP8 else BF16
WSCALE = 16.0 if USE_FP8 else 1.0
ONESV = 1.0
WPSCALE = 1.0
EXP_BIAS = -4.0 if USE_FP8 else 0.0
PT_F8 = False

NH = 8
NKV = 2
D = 64
QDIM = NH * D    # 512
KVDIM = NKV * D  # 128
NT = T // 128    # 16
NG = T // 256    # 8

_CACHE = {}


def _qsel(c, g):
    lo_t = max(2 * g, c)
    hi_t = min(2 * g + 1, c + 8)
    if lo_t > hi_t:
        return None
    return ((lo_t - 2 * g) * 128, (hi_t + 1 - 2 * g) * 128)


def _group_layout(g):
    """Union chunks of group g placed in 512-wide (2-slot) PSUM segment
    tiles.  placement[c] = (tile_idx, slot); chunk c's qtile-K columns
    live at slot*256 + K*128 of its tile.  Softmax denominators ride in
    the PV PSUM via a ones-column appended to v (no dn columns here)."""
    cs = list(range(max(0, 2 * g - 8), 2 * g + 2))
    n = len(cs)
    assert n % 2 == 0
    placement = {c: (i // 2, i % 2) for i, c in enumerate(cs)}
    ntiles = n // 2
    widths = [512] * ntiles
    # cl (and, for full windows, cf) contribute only one live 128-col
    # q-tile; pack them right after their partner chunk so the exp can
    # stop at 384 and the dead quarter is never touched.
    if n == 10:
        placement[cs[0]] = (0, 1)
        placement[cs[1]] = (0, 0)
        widths[0] = 384
    widths[-1] = 384
    return cs, placement, ntiles, widths


def build_program(fake_collective=False):
    nc = bacc.Bacc("TRN2", target_bir_lowering=False, debug=False,
                   enable_asserts=True, num_devices=8)

    xt_d = nc.dram_tensor("xt8", [128, 8, T], DT8, kind="ExternalInput")
    ve_d = nc.dram_tensor("ve_bf", [T, KVDIM], BF16, kind="ExternalInput")
    wq_d = nc.dram_tensor("wq8", [128, 8, QDIM], DT8, kind="ExternalInput")
    wkvz_d = nc.dram_tensor("wkvz8", [128, 8, 2 * KVDIM + NKV], DT8,
                            kind="ExternalInput")
    wp_d = nc.dram_tensor("wp8", [128, 8, 512], BF16, kind="ExternalInput")
    c2_d = nc.dram_tensor("c2_bf", [T, D], BF16, kind="ExternalInput")
    s2_d = nc.dram_tensor("s2_bf", [T, D], BF16, kind="ExternalInput")
    y_d = nc.dram_tensor("y_out", [T, 512], F32, kind="ExternalOutput")

    with TileContext(nc) as tc, nc.allow_low_precision(reason="fp8 pipeline"):
        with (
            tc.tile_pool(name="const", bufs=1) as constp,
            tc.tile_pool(name="persist", bufs=1) as pers,
            tc.tile_pool(name="work", bufs=3) as work,
            tc.tile_pool(name="probs", bufs=10) as probp,
            tc.tile_pool(name="att2", bufs=3) as att2,
            tc.tile_pool(name="ygsp", bufs=3) as ygsp,
            tc.tile_pool(name="p1ps", bufs=2, space="PSUM") as p1psp,
            tc.tile_pool(name="scps", bufs=2, space="PSUM") as scpsp,
            tc.tile_pool(name="b1ps", bufs=2, space="PSUM") as b1psp,
            tc.tile_pool(name="dram2", bufs=3, space="DRAM") as dram2p,
        ):
            # ---- constants ----
            ident = constp.tile([128, 128], BF16, name="ident")
            nc.gpsimd.memset(ident[:, :], 0.0)
            nc.gpsimd.affine_select(
                out=ident[:, :], in_=ident[:, :],
                compare_op=ALU.not_equal, fill=1.0, base=0,
                pattern=[[-1, 128]], channel_multiplier=1)
            # mask constants (rhs of identity-lhsT matmuls, direct
            # orientation psum[tk, tq] += M[tk, tq]):
            #   mm[:,   0:128] far  : -1e9 where tk <= tq
            #   mm[:, 128:384] all -1e9 (dead half-blocks)
            #   mm[:, 384:512] diag : -1e9 where tk > tq
            mm = constp.tile([128, 512], BF16, name="mm")
            nc.gpsimd.memset(mm[:, :], -1.0e9)
            nc.gpsimd.affine_select(
                out=mm[:, 0:128], in_=mm[:, 0:128],
                compare_op=ALU.is_ge, fill=0.0, base=0,
                pattern=[[1, 128]], channel_multiplier=-1)
            nc.gpsimd.affine_select(
                out=mm[:, 384:512], in_=mm[:, 384:512],
                compare_op=ALU.is_ge, fill=0.0, base=-1,
                pattern=[[-1, 128]], channel_multiplier=1)
            MFARF = mm[:, 0:256]
            MDIAGF = mm[:, 256:512]
            MDIAG = mm[:, 384:512]
            MFAR = mm[:, 0:128]
            ident8 = constp.tile([128, 128], DT8, name="ident8")
            nc.gpsimd.memset(ident8[:, :], 0.0)
            nc.gpsimd.affine_select(
                out=ident8[:, :], in_=ident8[:, :],
                compare_op=ALU.not_equal, fill=1.0, base=0,
                pattern=[[-1, 128]], channel_multiplier=1)
            magic = constp.tile([128, 1], U32, name="magic")
            nc.gpsimd.memset(magic[:, :], MAGIC)
            ebias = constp.tile([128, 1], F32, name="ebias")
            nc.gpsimd.memset(ebias[:, :], EXP_BIAS)

            # ---- weights / tables ----
            xt_sb = pers.tile([128, 8, T], DT8, name="xt_sb")
            wq_sb = pers.tile([128, 8, QDIM], DT8, name="wq_sb")
            wkvz_sb = pers.tile([128, 8, 2 * KVDIM + NKV], DT8,
                                name="wkvz_sb")
            wp_sb = pers.tile([128, 8, 512], BF16, name="wp_sb")
            c2_sb = pers.tile([128, NT, D], BF16, name="c2_sb")
            s2_sb = pers.tile([128, NT, D], BF16, name="s2_sb")
            ve_sb = pers.tile([128, NT, KVDIM], BF16, name="ve_sb")
            nc.sync.dma_start(xt_sb[:, :, 0:512], xt_d.ap()[:, :, 0:512])
            nc.sync.dma_start(wq_sb[:, :, :], wq_d.ap())
            nc.sync.dma_start(wkvz_sb[:, :, :], wkvz_d.ap())
            nc.sync.dma_start(c2_sb[:, :, :],
                              c2_d.ap().rearrange("(t p) d -> p t d", p=128))
            nc.sync.dma_start(s2_sb[:, :, :],
                              s2_d.ap().rearrange("(t p) d -> p t d", p=128))
            nc.sync.dma_start(ve_sb[:, :, :],
                              ve_d.ap().rearrange("(t p) d -> p t d", p=128))
            for tq in range(1, 4):
                nc.sync.dma_start(xt_sb[:, :, tq * 512:(tq + 1) * 512],
                                  xt_d.ap()[:, :, tq * 512:(tq + 1) * 512])
            nc.sync.dma_start(wp_sb[:, :, :], wp_d.ap())

            qT_all = pers.tile([128, 4, NT, 128], BF16, name="qT_all")
            kT_all = pers.tile([128, NT, 128], BF16, name="kT_all")
            # v tiles carry a trailing ones-column (col D) so the PV
            # matmul also accumulates the softmax denominator.
            v_sb2 = [pers.tile([128, 2, NKV, D + 1], BF16, name=f"v2_{cp}")
                     for cp in range(NT // 2)]
            for cp in range(NT // 2):
                nc.gpsimd.memset(v_sb2[cp][:, :, :, D:D + 1], ONESV)

            # ================= phase-1 tile emitter =================
            def emit_tile(t):
                part_b = emit_tile_a(t)
                part_b()

            def emit_tile_a(t):
                q_ps = p1psp.tile([128, QDIM], F32, tag="p1",
                                  name=f"q_ps{t}")
                kvz = p1psp.tile([128, KVDIM * 2 + NKV], F32, tag="p1",
                                 name=f"kvz{t}")
                for cc in range(0, 8, 2):
                    lhs = xt_sb[:, cc:cc + 2, t * 128:(t + 1) * 128]
                    st, sp_ = (cc == 0), (cc == 6)
                    if USE_FP8:
                        nc.tensor.matmul(q_ps[:, :], lhs,
                                         wq_sb[:, cc:cc + 2, :],
                                         start=st, stop=sp_, perf_mode=DR)
                        nc.tensor.matmul(kvz[:, :], lhs,
                                         wkvz_sb[:, cc:cc + 2, :],
                                         start=st, stop=sp_, perf_mode=DR)
                    else:
                        for o in range(2):
                            l1 = xt_sb[:, cc + o, t * 128:(t + 1) * 128]
                            s1_, p1_ = (cc + o == 0), (cc + o == 7)
                            nc.tensor.matmul(q_ps[:, :], l1,
                                             wq_sb[:, cc + o, :],
                                             start=s1_, stop=p1_)
                            nc.tensor.matmul(kvz[:, :], l1,
                                             wkvz_sb[:, cc + o, :],
                                             start=s1_, stop=p1_)

                # -- early kvz consumers so the next tile's k/v matmuls
                # aren't held back long --
                e_sb = work.tile([128, NKV], F32, tag="e_sb")
                nc.scalar.activation(e_sb[:, :],
                                     kvz[:, 2 * KVDIM:2 * KVDIM + NKV],
                                     AF.Exp, bias=0.0, scale=-1.0 / WSCALE)
                gp1 = work.tile([128, NKV], F32, tag="gp1")
                nc.vector.tensor_scalar_add(gp1[:, :], e_sb[:, :], 1.0)
                gr = work.tile([128, NKV], F32, tag="gr")
                nc.vector.reciprocal(gr[:, :], gp1[:, :])
                # v' = (ve2 * gate) + v   (ve2 = 2*WSCALE*ve host-side)
                vdst = v_sb2[t // 2]
                for u in range(NKV):
                    nc.vector.scalar_tensor_tensor(
                        vdst[:, t % 2, u, 0:D],
                        ve_sb[:, t, u * D:(u + 1) * D],
                        gr[:, u:u + 1],
                        kvz[:, KVDIM + u * D:KVDIM + (u + 1) * D],
                        ALU.mult, ALU.add)
                k_sb = work.tile([128, KVDIM], BF16, tag="k_sb")
                nc.vector.tensor_copy(k_sb[:, :], kvz[:, 0:KVDIM])
                q_sb = work.tile([128, QDIM], BF16, tag="q_sb")
                nc.scalar.copy(q_sb[:, :], q_ps[:, :])

                # -- mean-square sums (RoPE is orthogonal: pre-rotation).
                # Squares on ScalarE straight from PSUM; reduce on DVE. --
                ms = work.tile([128, NH + NKV], F32, tag="ms")
                sqq = work.tile([128, QDIM], BF16, tag="sqq")
                nc.scalar.square(sqq[:, :], q_ps[:, :])
                nc.vector.tensor_reduce(
                    ms[:, 0:NH],
                    sqq[:, :].rearrange("p (h d) -> p h d", h=NH),
                    AX.X, ALU.add)
                sqk = work.tile([128, KVDIM], BF16, tag="sqk")
                nc.scalar.square(sqk[:, :], kvz[:, 0:KVDIM])
                nc.vector.tensor_reduce(
                    ms[:, NH:NH + NKV],
                    sqk[:, :].rearrange("p (u d) -> p u d", u=NKV),
                    AX.X, ALU.add)

                # rs = rsqrt(ms + eps'): bit trick + 2 Newton iterations
                # (Sqrt on ScalarE would thrash the Exp act-func table)
                NR = NH + NKV
                a = work.tile([128, NR], F32, tag="a")
                nc.vector.tensor_scalar_add(a[:, :], ms[:, :],
                                            64.0 * EPS * WSCALE * WSCALE)
                s1 = work.tile([128, NR], U32, tag="s1")
                nc.vector.tensor_single_scalar(
                    s1[:, :], a[:, :].bitcast(U32), 1,
                    ALU.logical_shift_right)
                r0 = work.tile([128, NR], F32, tag="r0")
                nc.vector.tensor_tensor(
                    r0[:, :].bitcast(U32),
                    magic[:, :].to_broadcast((128, NR)).bitcast(U32),
                    s1[:, :], ALU.subtract)
                t1 = work.tile([128, NR], F32, tag="t1")
                rs = work.tile([128, NR], F32, tag="rs")
                for it in range(2):
                    src = r0 if it == 0 else rs
                    nc.vector.tensor_mul(t1[:, :], src[:, :], src[:, :])
                    nc.vector.tensor_mul(t1[:, :], t1[:, :], a[:, :])
                    nc.vector.tensor_scalar(t1[:, :], t1[:, :],
                                            -0.5, 1.5, ALU.mult, ALU.add)
                    nc.vector.tensor_mul(rs[:, :], src[:, :], t1[:, :])

                def part_b(t=t, q_sb=q_sb, k_sb=k_sb, rs=rs):
                    emit_tile_b(t, q_sb, k_sb, rs)
                return part_b

            def emit_tile_b(t, q_sb, k_sb, rs):
                # -- q rope: half-products on Pool, combine on DVE --
                qg = q_sb[:, :].rearrange("p (h s f) -> p h s f", h=NH, s=2)
                c2b = c2_sb[:, t].unsqueeze(1).broadcast_to([128, NH, D])
                s2b = s2_sb[:, t].rearrange("p (s f) -> p s f", s=2)
                m1 = work.tile([128, QDIM], BF16, tag="m1")
                m1g = m1[:, :].rearrange("p (h s f) -> p h s f", h=NH, s=2)
                nc.gpsimd.tensor_mul(
                    m1g[:, :, 0], qg[:, :, 1],
                    s2b[:, 0].unsqueeze(1).broadcast_to([128, NH, 32]))
                nc.gpsimd.tensor_mul(
                    m1g[:, :, 1], qg[:, :, 0],
                    s2b[:, 1].unsqueeze(1).broadcast_to([128, NH, 32]))
                rq = work.tile([128, QDIM], BF16, tag="rq")
                nc.vector.tensor_mul(
                    rq[:, :].rearrange("p (h d) -> p h d", h=NH),
                    q_sb[:, :].rearrange("p (h d) -> p h d", h=NH), c2b)
                nc.vector.tensor_add(rq[:, :], rq[:, :], m1[:, :])

                # q~ = rq * rs, head j -> column (j%4)*128 + (j//4)*64
                qn = work.tile([128, QDIM], BF16, tag="qn")
                nc.vector.tensor_tensor(
                    qn[:, :].rearrange("p (m hh d) -> p hh m d", m=4, hh=2),
                    rq[:, :].rearrange("p (hh m d) -> p hh m d", hh=2, m=4),
                    rs[:, 0:NH].rearrange("p (hh m) -> p hh m", hh=2)
                    .unsqueeze(3).broadcast_to([128, 2, 4, D]),
                    ALU.mult)
                nc.sync.dma_start_transpose(
                    out=qT_all[:, :, t, :], in_=qn[:, :])

                # -- k rope (Pool) --
                kg = k_sb[:, :].rearrange("p (u s f) -> p u s f", u=NKV, s=2)
                km = work.tile([128, KVDIM], BF16, tag="km")
                kmg = km[:, :].rearrange("p (u s f) -> p u s f", u=NKV, s=2)
                nc.gpsimd.tensor_mul(
                    kmg[:, :, 0], kg[:, :, 1],
                    s2b[:, 0].unsqueeze(1).broadcast_to([128, NKV, 32]))
                nc.gpsimd.tensor_mul(
                    kmg[:, :, 1], kg[:, :, 0],
                    s2b[:, 1].unsqueeze(1).broadcast_to([128, NKV, 32]))
                rk = work.tile([128, KVDIM], BF16, tag="rk")
                nc.gpsimd.tensor_mul(
                    rk[:, :].rearrange("p (u d) -> p u d", u=NKV),
                    k_sb[:, :].rearrange("p (u d) -> p u d", u=NKV),
                    c2_sb[:, t].unsqueeze(1).broadcast_to([128, NKV, D]))
                nc.gpsimd.tensor_add(rk[:, :], rk[:, :], km[:, :])
                # k~ = rk * rs_k (the 1/sqrt(d)=x8 scale rides on Exp)
                krs = work.tile([128, KVDIM], BF16, tag="krs")
                nc.vector.tensor_tensor(
                    krs[:, :].rearrange("p (u d) -> p u d", u=NKV),
                    rk[:, :].rearrange("p (u d) -> p u d", u=NKV),
                    rs[:, NH:NH + NKV].unsqueeze(2)
                    .broadcast_to([128, NKV, D]),
                    ALU.mult)
                nc.sync.dma_start_transpose(out=kT_all[:, t, :],
                                            in_=krs[:, :])

            # ================= phase-2 group machinery =================
            def emit_scores_tile(g, j, ti, cs, placement, widths, cf,
                                 cl, pts):
                base = (j // 4) * 64
                m = j % 4
                sc = scpsp.tile([128, 512], F32, tag="sc",
                                name=f"sc{g}_{j}_{ti}")
                for c in cs:
                    tic, s = placement[c]
                    if tic != ti:
                        continue
                    o = s * 256
                    if c == cf:
                        nc.tensor.matmul(sc[:, o:o + 128], ident[:, :],
                                         MFAR, start=True, stop=False,
                                         skip_group_check=True)
                        nc.tensor.matmul(
                            sc[:, o:o + 128],
                            kT_all[base:base + 64, c, :],
                            qT_all[base:base + 64, m, 2 * g, :],
                            start=False, stop=True, skip_group_check=True)
                    elif c == cl:
                        nc.tensor.matmul(sc[:, o:o + 128], ident[:, :],
                                         MDIAG, start=True, stop=False,
                                         skip_group_check=True)
                        nc.tensor.matmul(
                            sc[:, o:o + 128],
                            kT_all[base:base + 64, c, :],
                            qT_all[base:base + 64, m, 2 * g + 1, :],
                            start=False, stop=True, skip_group_check=True)
                    else:
                        diag = (c == 2 * g)
                        far1 = (c == 2 * g - 7)
                        nc.tensor.matmul(
                            sc[:, o:o + 256],
                            kT_all[base:base + 64, c, :],
                            qT_all[base:base + 64, m, 2 * g:2 * g + 2, :],
                            start=True, stop=not (diag or far1),
                            skip_group_check=True)
                        if diag:
                            nc.tensor.matmul(
                                sc[:, o:o + 128], ident[:, :], MDIAG,
                                start=False, stop=True,
                                skip_group_check=True)
                        if far1:
                            nc.tensor.matmul(
                                sc[:, o + 128:o + 256], ident[:, :], MFAR,
                                start=False, stop=True,
                                skip_group_check=True)
                pt = probp.tile([128, 512], BF16, tag="pt",
                                name=f"pt{g}_{j}_{ti}")
                nc.scalar.activation(pt[:, 0:widths[ti]],
                                     sc[:, 0:widths[ti]],
                                     AF.Exp, bias=ebias[:, :], scale=8.0)
                pts.setdefault(j, []).append(pt)

            def emit_pv(g, j, cs, placement, pts, yt8):
                u = j // 4
                jj = j % 4
                for K in range(2):
                    t = 2 * g + K
                    ck = [c for c in cs if max(0, t - 8) <= c <= t]
                    ops = []
                    i = 0
                    while i < len(ck):
                        c = ck[i]
                        if PT_F8 and c % 2 == 0 and i + 1 < len(ck):
                            ops.append((True, c))
                            i += 2
                        else:
                            ops.append((False, c))
                            i += 1
                    nops = len(ops)
                    dst = yt8[K][u][:, jj * (D + 1):(jj + 1) * (D + 1)]
                    for oi, (paired, c) in enumerate(ops):
                        st, sp_ = (oi == 0), (oi == nops - 1)
                        ti, s = placement[c]
                        pt = pts[j][ti]
                        if paired:
                            lhs = pt[:, :].rearrange(
                                "p (s q) -> p s q", s=2)[
                                :, :, K * 128:(K + 1) * 128]
                            nc.tensor.matmul(
                                dst, lhs,
                                v_sb2[c // 2][:, :, u, :],
                                start=st, stop=sp_, perf_mode=DR,
                                skip_group_check=True)
                        else:
                            # cl packs its single live q-tile at o+0
                            o = (s % 2) * 256 + (0 if c == 2 * g + 1
                                                 else K * 128)
                            lhs = pt[:, o:o + 128]
                            nc.tensor.matmul(
                                dst, lhs,
                                v_sb2[c // 2][:, c % 2, u, :],
                                start=st, stop=sp_, skip_group_check=True)

            def emit_finish(g, yt8):
                """normalize + transpose + exchange for group g; returns
                a closure emitting the projection (deferred so the
                exchange round-trip hides under the next group)."""
                rec = att2.tile([128, 2, 2, 4], F32, tag="rec",
                                name=f"rec{g}")
                for K in range(2):
                    for u in range(2):
                        nc.vector.reciprocal(
                            rec[:, K, u, :],
                            yt8[K][u][:, :].rearrange(
                                "p (h e) -> p h e", e=D + 1)[:, :, D])
                agin = dram2p.tile([8, 128, 128], BF16, tag="agin")
                agout = dram2p.tile([16, 128, 128], BF16, tag="agout")
                for K in range(2):
                    yn = att2.tile([128, QDIM], BF16, tag="yn",
                                   name=f"yn{g}_{K}")
                    for u in range(2):
                        nc.vector.tensor_tensor(
                            yn[:, u * 256:(u + 1) * 256].rearrange(
                                "p (h d) -> p h d", h=4),
                            yt8[K][u][:, :].rearrange(
                                "p (h e) -> p h e", e=D + 1)[:, :, 0:D],
                            rec[:, K, u, :].unsqueeze(2)
                            .broadcast_to([128, 4, D]),
                            ALU.mult)
                    ytp = b1psp.tile([128, QDIM], BF16, tag="b1",
                                     bufs=4, name=f"ytp{g}_{K}")
                    for bl in range(4):
                        nc.tensor.transpose(
                            ytp[:, bl * 128:(bl + 1) * 128],
                            yn[:, bl * 128:(bl + 1) * 128], ident[:, :])
                    yto = att2.tile([128, QDIM], BF16, tag="yto",
                                    name=f"yto{g}_{K}")
                    nc.vector.tensor_copy(yto[:, :], ytp[:, :])
                    nc.sync.dma_start(
                        agin[K * 4:(K + 1) * 4, :, :].rearrange(
                            "b p n -> p b n"),
                        yto[:, :].rearrange("p (b n) -> p b n", b=4))
                if fake_collective:
                    nc.sync.dma_start(agout[0:8], agin[:, :, :])
                    nc.sync.dma_start(agout[8:16], agin[:, :, :])
                else:
                    nc.gpsimd.collective_compute(
                        "AllGather", ALU.bypass,
                        replica_groups=[[0, 1], [2, 3], [4, 5], [6, 7]],
                        ins=[agin[:, :, :].opt()],
                        outs=[agout[:, :, :].opt()])
                ygs = ygsp.tile([128, 16, 128], BF16, tag="ygs",
                                name=f"ygs{g}")
                nc.sync.dma_start(
                    ygs[:, :, :],
                    agout[:, :, :].rearrange("c p n -> p c n"))

                def proj():
                    o2 = att2.tile([128, 2, 512], F32, tag="o2",
                                   name=f"o2_{g}")
                    for K in range(2):
                        # chunk order in ygs: [own t0 | own t1 | peer t0
                        # | peer t1], 4 blocks each
                        idx = [K * 4, K * 4 + 8]
                        pr = scpsp.tile([128, 512], F32, tag="sc",
                                        name=f"pr{g}_{K}")
                        if PROJ_F8:
                            for hi, i0 in enumerate(idx):
                                for c2i in range(0, 4, 2):
                                    nc.tensor.matmul(
                                        pr[:, :],
                                        ygs[:, i0 + c2i:i0 + c2i + 2, :],
                                        wp_sb[:, hi * 4 + c2i:
                                              hi * 4 + c2i + 2, :],
                                        start=(hi == 0 and c2i == 0),
                                        stop=(hi == 1 and c2i == 2),
                                        perf_mode=DR,
                                        skip_group_check=True)
                        else:
                            for hi, i0 in enumerate(idx):
                                for c1 in range(4):
                                    nc.tensor.matmul(
                                        pr[:, :], ygs[:, i0 + c1, :],
                                        wp_sb[:, hi * 4 + c1, :],
                                        start=(hi == 0 and c1 == 0),
                                        stop=(hi == 1 and c1 == 3),
                                        skip_group_check=True)
                        nc.vector.tensor_copy(o2[:, K, :], pr[:, :])
                    nc.sync.dma_start(
                        y_d.ap()[g * 256:(g + 1) * 256, :].rearrange(
                            "(i p) n -> p i n", p=128),
                        o2[:, :, :])
                return proj

            # ================= interleaved schedule =================
            for t in range(8):
                emit_tile(t)
            deferred_proj = None
            for g in range(NG):
                cs, placement, ntiles, widths = _group_layout(g)
                cf = cs[0] if _qsel(cs[0], g) == (0, 128) else None
                cl = cs[-1]
                yt8 = [[b1psp.tile([128, 4 * (D + 1)], F32, tag="b1",
                                   bufs=4, name=f"yt8_{g}_{K}_{u}")
                        for u in range(2)] for K in range(2)]
                pts = {}
                for j in range(NH):
                    emit_scores_tile(g, j, 0, cs, placement, widths,
                                     cf, cl, pts)
                    if j > 0:
                        emit_pv(g, j - 1, cs, placement, pts, yt8)
                    for ti in range(1, ntiles):
                        emit_scores_tile(g, j, ti, cs, placement, widths,
                                         cf, cl, pts)
                    if j == 5 and deferred_proj is not None:
                        deferred_proj()
                        deferred_proj = None
                    if j == 1 and 2 * g + 8 < NT:
                        emit_tile(2 * g + 8)
                    if j == 3 and 2 * g + 9 < NT:
                        emit_tile(2 * g + 9)
                emit_pv(g, NH - 1, cs, placement, pts, yt8)
                deferred_proj = emit_finish(g, yt8)
            deferred_proj()

    nc.compile()
    return nc


def _prep_inputs(x, ve, cos, sin, wq, wk, wv, wproj, wgate):
    bf = ml_dtypes.bfloat16
    f8 = ml_dtypes.float8_e4m3
    w8 = f8 if USE_FP8 else bf
    cosf = np.asarray(cos, np.float32).reshape(T, 32)
    sinf = np.asarray(sin, np.float32).reshape(T, 32)
    c2 = np.concatenate([cosf, cosf], axis=1).astype(bf)
    s2 = np.concatenate([sinf, -sinf], axis=1).astype(bf)
    x = np.asarray(x, np.float32)
    ve = np.asarray(ve, np.float32)
    wq = np.asarray(wq, np.float32) * WSCALE
    wk = np.asarray(wk, np.float32) * WSCALE
    wv = np.asarray(wv, np.float32) * WSCALE
    wproj = np.asarray(wproj, np.float32) * WPSCALE
    wgate = np.asarray(wgate, np.float32) * WSCALE

    def wfmt(w):  # [1024, N] -> [128, 8, N]
        return np.ascontiguousarray(
            w.reshape(8, 128, -1).transpose(1, 0, 2)).astype(w8)

    maps = []
    for core in range(8):
        b, hp = core // 2, core % 2
        xt8 = np.ascontiguousarray(
            x[b].T.reshape(8, 128, T).transpose(1, 0, 2)).astype(w8)
        wgp = np.zeros((1024, NKV), np.float32)
        wgp[0:8] = wgate[:, hp * 2:(hp + 1) * 2]
        wkvz = np.concatenate([wk[:, hp * 128:(hp + 1) * 128],
                               wv[:, hp * 128:(hp + 1) * 128], wgp], axis=1)
        maps.append({
            "xt8": xt8,
            "ve_bf": (2.0 * WSCALE
                      * ve[b][:, hp * 128:(hp + 1) * 128]).astype(bf),
            "wq8": wfmt(wq[:, hp * 512:(hp + 1) * 512]),
            "wkvz8": wfmt(wkvz),
            "wp8": np.ascontiguousarray(
                wproj[:, hp * 512:(hp + 1) * 512].reshape(8, 128, -1)
                .transpose(1, 0, 2)).astype(bf),
            "c2_bf": c2,
            "s2_bf": s2,
        })
    return maps


def kernel(x, ve, cos, sin, wq, wk, wv, wproj, wgate, window):
    assert int(window) == WINDOW
    if "nc" not in _CACHE:
        _CACHE["nc"] = build_program()
    nc = _CACHE["nc"]
    maps = _prep_inputs(x, ve, cos, sin, wq, wk, wv, wproj, wgate)
    res = run_bass_kernel_spmd(nc, maps, list(range(8))).results
    y = np.empty((B, T, C), np.float32)
    for core in range(8):
        b, hp = core // 2, core % 2
        y[b][:, hp * 512:(hp + 1) * 512] = res[core]["y_out"]
    return y

